# revision 1
# baseline (speedup 1.0000x reference)
"""CapsuleTransformConv on 8 Trainium2 NeuronCores.

Problem:  x [4,16,16,32,16] f32, matrix [288,16,512] f32.
          im2col (K=3, VALID) -> tile [4,14,14,288,16]
          votes  = einsum('bhwna,nac->bhwnc', tile, matrix)
          out    = votes.reshape(4,14,14,288,32,16)

Sharding: tensor-parallel over the filter*atom output axis (512 -> 64 per
core).  Every core reads the full x (2 MB) and its 64-wide slice of the
weights; writes its [784, 288, 64] slice of the output (~58 MB, the
dominant HBM traffic).

Per-core kernel (~253 us HW, vs ~208 us pure write time at the measured
~290 GB/s per-core effective HBM write rate with all 8 cores active):
  - x is loaded once (2 DMAs) and PE-transposed into 4 per-octet tiles
    xT[(c_in_octet, atom)=128 partitions, (b,h,w)=1024]; x is read from
    HBM exactly once.
  - Per tap (ki,kj), GPSIMD compacts the im2col gather into
    tap[(dc,a), oct*784 + (b,i,j)] so every matmul's stationary operand
    is a flat contiguous slice (walrus requires a single free dim).
  - Weights for 8 consecutive capsules (one c-octet of one tap) are laid
    out block-diagonally in a [128, 512] f32r tile so one K=128 matmul
    computes 8 independent [pos,16]@[16,64] capsule matmuls.  FP32r
    matmul inputs must be produced by a rounding instruction (never by
    DMA), so paint DMAs land in a reused memset-once f32 buffer and a
    full-partition DVE copy rounds each 4-group chunk into its per-tap
    wpack tile.
  - Main loop: 9 taps x (4 batches x 2 i-windows); each iteration runs
    4 matmuls (c-octets) into one 4-bank PSUM tile, a PSUM->SBUF copy
    split by bank pairs across Vector||Scalar, and one contiguous
    0.7-0.9 MB DMA to the tap-major output, alternating the two HWDGE
    rings.
  - Matmuls run in float32r (TF32-class, 1 cyc/row vs 4 for fp32);
    fp32 accumulation in PSUM; rel err vs fp32 reference ~1.7e-4.
    Set MM_MODE="f32" for bit-exact output at ~303 us.
"""

import numpy as np

B, H, W, C, A = 4, 16, 16, 32, 16
KS = 3
OH = OW = 14
NCAP = KS * KS * C          # 288 capsules
FTOT = 512                  # filter*atom
NCORES = 8
FPC = FTOT // NCORES        # 64 output features per core
POS = B * OH * OW           # 784 output positions
NG = NCAP // 8              # 36 groups of 8 capsules = (tap, c-octet)

_NC_CACHE = {}
MM_MODE = "f32r"  # "f32" (exact, 4 cyc/row) or "f32r" (TF32-class, 1 cyc/row)


def _build_nc(mm_f32r=True):
    import concourse.bass as bass  # noqa: F401
    import concourse.mybir as mybir
    import concourse.tile as tile
    from concourse import bacc, masks

    f32 = mybir.dt.float32
    mmdt = mybir.dt.float32r if mm_f32r else mybir.dt.float32

    nc = bacc.Bacc(None, target_bir_lowering=False)
    x_d = nc.declare_dram_parameter("x", [B, H, W, C, A], f32, isOutput=False)
    m_d = nc.declare_dram_parameter("mat", [NCAP, A, FPC], f32, isOutput=False)
    # Tap-major output layout: out[kk, pos, 32*64].  Each inner-loop DMA then
    # writes one fully contiguous ~0.7-0.9 MB block (vs 8 KB runs strided by
    # 72 KB in pos-major layout); the host transposes kk back into n.
    o_d = nc.declare_dram_parameter("out", [KS * KS, POS, 32 * FPC], f32,
                                    isOutput=True)

    x2d = x_d.rearrange("b h w c a -> (b h w) (c a)")   # [1024, 512]

    with tile.TileContext(nc) as tc:
        with (
            tc.tile_pool(name="const", bufs=1) as constp,
            tc.tile_pool(name="big", bufs=1) as bigp,
            tc.tile_pool(name="stage", bufs=3) as stagep,
            tc.tile_pool(name="tapp", bufs=2) as tapp,
            tc.tile_pool(name="psum", bufs=2, space="PSUM") as psump,
        ):
            ident = constp.tile([128, 128], f32, tag="ident")
            masks.make_identity(nc, ident[:])

            # ---- weights chunk 0 paint: first in the sync ring ----
            # (moved ahead of the x loads; see the wpack build below)
            msrc = m_d.rearrange("(g gc) a f -> gc a g f", gc=8)
            wtmp = bigp.tile([128, 16 * 512], f32, tag="wtmp")
            # Small memset on DVE (idle, early) so round-0 paints go first.
            nc.vector.memset(wtmp[:, 0:2048], 0.0)
            nc.gpsimd.memset(wtmp[:, 2048:], 0.0)
            wtv = wtmp[:].rearrange("p (g v) -> p g v", g=16)
            for gc in range(8):
                nc.sync.dma_start(
                    wtv[gc * 16:(gc + 1) * 16, 0:4, gc * FPC:(gc + 1) * FPC],
                    msrc[gc, :, 0:4, :],
                )

            # ---- x: HBM -> SBUF once, four 2-slab tiles [128, 2, 512] ----
            # (per-batch granularity: batch b's transposes depend only on
            # tile b, so the first matmul chain starts ~2us after the first
            # 512 KB lands)
            xsrc = x2d.rearrange("(t s p) c -> t p s c", t=4, p=128)
            x_sbs = [
                bigp.tile([128, 2 * 512], f32, tag=f"x_sb{t}", name=f"x_sb{t}")
                for t in range(4)
            ]
            for t in range(4):
                nc.sync.dma_start(
                    x_sbs[t][:].rearrange("p (s c) -> p s c", s=2), xsrc[t]
                )

            # ---- weights: block-diagonal wpack, built per-tap ----
            # wpack_c[(gc,a), oct*512 + gc*64 + f] = matrix[(c*4+oct)*8+gc, a, f]
            # else 0.  FP32r matmul inputs must be produced by a rounding
            # instruction (never by DMA), so paint DMAs land in transient f32
            # tiles and a full-partition engine copy rounds each chunk.
            # One chunk per tap kk so kk=0 matmuls start without waiting for
            # the whole weight build.  The two transient tiles are memset
            # once: every chunk paints the same diagonal positions, so the
            # off-diagonal zeros stay clean across reuse.
            # One serially-reused paint buffer covering 4 taps (16 groups);
            # every round paints the same diagonal positions, so the memset
            # zeros stay clean across reuse.  Round 0 (tap 0) was painted
            # above, ahead of the x loads.
            wpacks = []
            for rnd, ntap in ((0, 1), (1, 4), (2, 4)):
                g0 = (0, 4, 20)[rnd]  # first group of this round
                ng = ntap * 4
                if rnd > 0:
                    for gc in range(8):
                        # Scalar ring: idle until outputs begin.
                        nc.scalar.dma_start(
                            wtv[gc * 16:(gc + 1) * 16, 0:ng,
                                gc * FPC:(gc + 1) * FPC],
                            msrc[gc, :, g0: g0 + ng, :],
                        )
                for t in range(ntap):
                    kk_of = g0 // 4 + t
                    wp = bigp.tile(
                        [128, 4 * 512], mmdt,
                        tag=f"wpack{kk_of}", name=f"wpack{kk_of}",
                    )
                    nc.vector.tensor_copy(
                        wp[:], wtmp[:, t * 2048:(t + 1) * 2048]
                    )
                    wpacks.append(wp)

            # ---- xT: PE-transpose x into 4 per-octet tiles [(dc,a), (b,h,w)]
            # Separate tiles so each octet's im2col cast can start as soon as
            # its own 8 transposes land.
            xts = [
                bigp.tile([128, 1024], f32, tag=f"xt{o}", name=f"xt{o}")
                for o in range(4)
            ]
            for s in range(8):
                for oct in range(4):
                    tr = psump.tile([128, 128], f32, tag="mm")
                    nc.tensor.transpose(
                        tr[:],
                        x_sbs[s // 2][
                            :, (s % 2) * 512 + oct * 128:
                            (s % 2) * 512 + (oct + 1) * 128
                        ],
                        ident[:],
                    )
                    dst = xts[oct][:, s * 128:(s + 1) * 128]
                    if (s + oct) % 2 == 0:
                        nc.vector.tensor_copy(dst, tr[:])
                    else:
                        nc.scalar.copy(dst, tr[:])

            xtvs = [
                t[:].rearrange("p (b h w) -> p b h w", b=B, h=H) for t in xts
            ]

            # ---- main loop: 9 taps (outer) x per-batch pos windows ----
            # The matmul stationary operand must be a single flat free dim
            # (walrus constraint), so per tap we compact the im2col gather
            # into tap[(dc,a), oct*784 + (b,i,j)] with GPSIMD copies.
            it = 0
            for kk in range(9):
                ki, kj = kk // 3, kk % 3
                tap = tapp.tile([128, 4 * POS], mmdt, tag="tap")
                for oct in range(4):
                    dst = tap[:, oct * POS:(oct + 1) * POS].rearrange(
                        "p (b i j) -> p b i j", b=B, i=OH
                    )
                    src = xtvs[oct][:, :, ki: ki + OH, kj: kj + OW]
                    if kk == 0:
                        # First tap per-batch on DVE/ACT (idle at startup):
                        # batch b's cast only needs x slabs 2b..2b+1, so the
                        # first matmul starts as soon as the first slabs
                        # transpose.  Later taps prefetch on idle GPSIMD.
                        for bb in range(B):
                            if (oct + bb) % 2 == 0:
                                nc.vector.tensor_copy(
                                    dst[:, bb], src[:, bb]
                                )
                            else:
                                nc.scalar.copy(dst[:, bb], src[:, bb])
                    else:
                        nc.gpsimd.tensor_copy(dst, src)
                for b in range(B):
                    for i0, ni in ((0, 8), (8, 6)):
                        m = ni * OW  # 112 or 84 output positions
                        ps = psump.tile([128, 2048], f32, tag="mm")
                        for oct in range(4):
                            off = oct * POS + b * (OH * OW) + i0 * OW
                            nc.tensor.matmul(
                                ps[0:m, oct * 512:(oct + 1) * 512],
                                tap[:, off: off + m],
                                wpacks[kk][:, oct * 512:(oct + 1) * 512],
                                start=True,
                                stop=True,
                            )
                        st = stagep.tile([128, 2048], f32, tag="st")
                        # Split the PSUM->SBUF copy by bank pairs so DVE and
                        # ACT run in parallel (different PSUM banks).
                        nc.vector.tensor_copy(st[0:m, 0:1024], ps[0:m, 0:1024])
                        nc.scalar.copy(st[0:m, 1024:2048], ps[0:m, 1024:2048])
                        # Alternate the two HWDGE rings (SP / ACT) so output
                        # DMAs pipeline across both.
                        dma_eng = nc.sync if it % 2 == 0 else nc.scalar
                        q0 = b * (OH * OW) + i0 * OW
                        dma_eng.dma_start(
                            o_d[kk, q0: q0 + m, :],
                            st[0:m, :],
                        )
                        it += 1

    nc.compile()
    return nc


def _get_nc():
    key = MM_MODE
    if key not in _NC_CACHE:
        _NC_CACHE[key] = _build_nc(mm_f32r=(MM_MODE == "f32r"))
    return _NC_CACHE[key]


def kernel(x, matrix):
    from concourse.bass_utils import run_bass_kernel_spmd

    x = np.ascontiguousarray(x, dtype=np.float32)
    matrix = np.ascontiguousarray(matrix, dtype=np.float32)
    nc = _get_nc()
    in_maps = [
        {
            "x": x,
            "mat": np.ascontiguousarray(matrix[:, :, c * FPC:(c + 1) * FPC]),
        }
        for c in range(NCORES)
    ]
    r = run_bass_kernel_spmd(nc, in_maps, list(range(NCORES)))
    # parts[c]: [9, 784, 2048] tap-major -> [784, kk, 32, core, 64] -> full
    arr = np.stack([r.results[c]["out"] for c in range(NCORES)])
    arr = arr.reshape(NCORES, KS * KS, POS, 32, FPC)
    arr = arr.transpose(2, 1, 3, 0, 4)               # [pos, kk, 32, core, f]
    full = arr.reshape(POS, NCAP, FTOT)
    return np.ascontiguousarray(
        full.reshape(B, OH, OW, NCAP, 32, 16).astype(np.float32)
    )



# revision 5
# speedup vs baseline: 1.9634x; 1.9634x over previous
"""CapsuleTransformConv on 8 Trainium2 NeuronCores.

Problem:  x [4,16,16,32,16] f32, matrix [288,16,512] f32.
          im2col (K=3, VALID) -> tile [4,14,14,288,16]
          votes  = einsum('bhwna,nac->bhwnc', tile, matrix)
          out    = votes.reshape(4,14,14,288,32,16)

Sharding: tensor-parallel over the filter*atom output axis (512 -> 64 per
core).  Every core gets the fp16 x (1 MB) and a host-packed fp16
block-diagonal stationary-weight image (4.6 MB); it writes its
[784, 288, 64] output slice in fp16 (~29 MB, the dominant HBM traffic).
The rel-err budget is 2e-2; fp16 inputs + f32 PSUM accumulate + fp16
output land at ~1e-3.

Per-core kernel:
  - x is loaded once (4 DMAs, fp16) and PE-transposed into 4 per-octet
    tiles xts[oct][(dc,a)=128, (b,h,w)=1024].
  - Weights are packed ON THE HOST into 144 stationary blocks
    [K=(dc,a)=128, M=128]: block t=(kk,oct,fb) holds capsules
    n=kk*32+oct*8+{2fb,2fb+1} block-diagonally (rows dc*16..dc*16+16
    nonzero for column group dc-2fb).  One clean DMA brings them in; the
    kk=0 chunk is fetched first so matmuls start early.
  - Matmul orientation: weights are the STATIONARY operand (128-col
    LDWEIGHTS, amortized over two 392-row moving windows); the moving
    operand streams the im2col window directly from xts via a strided
    3-free-dim AP [2 batches, 14, 14] -- no per-tap im2col compaction at
    all (the baseline burned ~97us of GPSIMD on that).
  - fp16 matmul = 1 cyc/row on the PE; f32 accumulate in PSUM.
  - PSUM->SBUF copies (f32 -> fp16 cast) rotate over DVE/ACT/GPSIMD;
    one contiguous ~800 KB output DMA per (kk,oct), alternating the
    sync HWDGE ring and the gpsimd SWDGE path.
  - Host: upcast fp16 -> f32 + transpose into the reference layout.
"""

import numpy as np

B, H, W, C, A = 4, 16, 16, 32, 16
KS = 3
OH = OW = 14
NCAP = KS * KS * C          # 288 capsules
FTOT = 512                  # filter*atom
NCORES = 8
FPC = FTOT // NCORES        # 64 output features per core
POS = B * OH * OW           # 784 output positions
NPOS = B * H * W            # 1024 input positions
NBLK = KS * KS * 4 * 4      # 144 stationary blocks (kk, oct, fb)

_NC_CACHE = {}


def _build_nc():
    import concourse.mybir as mybir
    import concourse.tile as tile
    from concourse import bacc, masks

    f16 = mybir.dt.float16
    f32 = mybir.dt.float32

    nc = bacc.Bacc(None, target_bir_lowering=False)
    x_d = nc.declare_dram_parameter("x", [NPOS, 512], f16, isOutput=False)
    w_d = nc.declare_dram_parameter("wp", [128, NBLK * 128], f16,
                                    isOutput=False)
    # out[kk, oct, col(=hi*64+f), fb, pos] fp16; host decodes
    # n = kk*32 + oct*8 + 2*fb + hi, feature = core*64 + f.
    o_d = nc.declare_dram_parameter("out", [KS * KS, 4, 128, 4, POS], f16,
                                    isOutput=True)

    with tile.TileContext(nc) as tc:
        with (
            tc.tile_pool(name="const", bufs=1) as constp,
            tc.tile_pool(name="big", bufs=1) as bigp,
            tc.tile_pool(name="stage", bufs=3) as stagep,
            tc.tile_pool(name="pst", bufs=2, space="PSUM") as pstp,
            tc.tile_pool(name="psum", bufs=6, space="PSUM") as psump,
        ):
            ident = constp.tile([128, 128], f16, tag="ident")
            masks.make_identity(nc, ident[:])

            # ---- weights: kk=0 chunk first so the MM stream starts early
            wsbA = bigp.tile([128, 16 * 128], f16, tag="wsbA")
            nc.scalar.dma_start(wsbA[:], w_d[:, 0:16 * 128])

            # ---- x: HBM -> SBUF, 4 tiles of 256 positions each
            xsrc = x_d.rearrange("(t s p) c -> t p s c", t=4, p=128)
            x_sbs = [
                bigp.tile([128, 2 * 512], f16, tag=f"x_sb{t}",
                          name=f"x_sb{t}")
                for t in range(4)
            ]
            for t in range(4):
                nc.sync.dma_start(
                    x_sbs[t][:].rearrange("p (s c) -> p s c", s=2), xsrc[t]
                )

            # rest of the weights, split in two so kk=1..4 lands sooner
            wsbB = bigp.tile([128, 64 * 128], f16, tag="wsbB")
            wsbC = bigp.tile([128, 64 * 128], f16, tag="wsbC")
            nc.scalar.dma_start(wsbB[:], w_d[:, 16 * 128:80 * 128])
            nc.scalar.dma_start(wsbC[:], w_d[:, 80 * 128:144 * 128])

            def wblock(t):
                if t < 16:
                    return wsbA[:, t * 128:(t + 1) * 128]
                if t < 80:
                    return wsbB[:, (t - 16) * 128:(t - 15) * 128]
                return wsbC[:, (t - 80) * 128:(t - 79) * 128]

            # ---- xT: PE-transpose x into 4 per-octet tiles [(dc,a), pos]
            xts = [
                bigp.tile([128, NPOS], f16, tag=f"xt{o}", name=f"xt{o}")
                for o in range(4)
            ]
            for oct in range(4):
                for s in range(8):
                    tr = pstp.tile([128, 128], f16, tag="tr")
                    nc.tensor.transpose(
                        tr[:],
                        x_sbs[s // 2][
                            :, (s % 2) * 512 + oct * 128:
                            (s % 2) * 512 + (oct + 1) * 128
                        ],
                        ident[:],
                    )
                    dst = xts[oct][:, s * 128:(s + 1) * 128]
                    if (s + oct) % 2 == 0:
                        nc.vector.tensor_copy(dst, tr[:])
                    else:
                        nc.scalar.copy(dst, tr[:])

            xtvs = [
                t[:].rearrange("p (b h w) -> p b h w", b=B, h=H) for t in xts
            ]

            # ---- main stream: 144 stationary blocks x 2 moving windows
            HB = 2 * OH * OW  # 392 positions per half (2 batches)
            cp_rot = 0
            it = 0
            for kk in range(KS * KS):
                ki, kj = kk // 3, kk % 3
                for oct in range(4):
                    st = stagep.tile([128, 4 * POS], f16, tag="st")
                    for fb in range(4):
                        t = (kk * 4 + oct) * 4 + fb
                        for half in range(2):
                            ps = psump.tile([128, 512], f32, tag="mm")
                            mv = xtvs[oct][
                                :, 2 * half:2 * half + 2,
                                ki:ki + OH, kj:kj + OW,
                            ]
                            nc.tensor.matmul(
                                ps[:, 0:HB], wblock(t), mv,
                                start=True, stop=True,
                            )
                            dst = st[:, fb * POS + half * HB:
                                     fb * POS + (half + 1) * HB]
                            # alternate PSUM->SBUF cast copies DVE/ACT
                            # (GPSIMD cannot access PSUM)
                            if cp_rot % 2 == 0:
                                nc.vector.tensor_copy(dst, ps[:, 0:HB])
                            else:
                                nc.scalar.copy(dst, ps[:, 0:HB])
                            cp_rot += 1
                    dma_eng = nc.sync if it % 2 == 0 else nc.gpsimd
                    dma_eng.dma_start(
                        o_d[kk, oct],
                        st[:].rearrange("p (fb q) -> p fb q", fb=4),
                    )
                    it += 1

    nc.compile()
    return nc


def _get_nc():
    if "nc" not in _NC_CACHE:
        _NC_CACHE["nc"] = _build_nc()
    return _NC_CACHE["nc"]


def _in_maps(x, matrix):
    """Full f32 inputs -> per-core fp16 input dicts."""
    x16 = np.ascontiguousarray(
        x.reshape(NPOS, 512).astype(np.float16)
    )
    m16 = matrix.astype(np.float16)  # [288, 16, 512]
    maps = []
    for c in range(NCORES):
        msl = m16[:, :, c * FPC:(c + 1) * FPC]      # [288, 16, 64]
        blk = msl.reshape(KS * KS, 4, 4, 2, 16, FPC)  # [kk,oct,fb,hi,a,f]
        wp = np.zeros((128, NBLK, 128), np.float16)
        for fb in range(4):
            for hi in range(2):
                dc = 2 * fb + hi
                wp[dc * 16:(dc + 1) * 16].reshape(16, KS * KS, 4, 4, 128)[
                    :, :, :, fb, hi * FPC:(hi + 1) * FPC
                ] = blk[:, :, fb, hi].transpose(2, 0, 1, 3)
        maps.append({
            "x": x16,
            "wp": np.ascontiguousarray(wp.reshape(128, NBLK * 128)),
        })
    return maps


def kernel(x, matrix):
    from concourse.bass_utils import run_bass_kernel_spmd

    x = np.ascontiguousarray(x, dtype=np.float32)
    matrix = np.ascontiguousarray(matrix, dtype=np.float32)
    nc = _get_nc()
    r = run_bass_kernel_spmd(nc, _in_maps(x, matrix), list(range(NCORES)))
    # parts[c]: [9, 4, 128, 4, 784] fp16
    arr = np.stack([r.results[c]["out"] for c in range(NCORES)])
    arr = arr.reshape(NCORES, KS * KS, 4, 2, FPC, 4, POS)
    # [core, kk, oct, hi, f, fb, pos] -> [pos, kk, oct, fb, hi, core, f]
    arr = arr.transpose(6, 1, 2, 5, 3, 0, 4)
    full = arr.reshape(POS, NCAP, FTOT).astype(np.float32)
    return np.ascontiguousarray(
        full.reshape(B, OH, OW, NCAP, 32, 16)
    )


# revision 6
# speedup vs baseline: 2.1915x; 1.1161x over previous
"""CapsuleTransformConv on 8 Trainium2 NeuronCores.

Problem:  x [4,16,16,32,16] f32, matrix [288,16,512] f32.
          im2col (K=3, VALID) -> tile [4,14,14,288,16]
          votes  = einsum('bhwna,nac->bhwnc', tile, matrix)
          out    = votes.reshape(4,14,14,288,32,16)

Sharding: tensor-parallel over the filter*atom output axis (512 -> 64 per
core).  Every core gets the fp16 x (1 MB) and a host-packed compact fp16
stationary-weight image (1.2 MB); it writes its [784, 288, 64] output
slice in fp16 (~29 MB, the dominant HBM traffic).  The rel-err budget is
2e-2; fp16 inputs + f32 PSUM accumulate + fp16 output land at ~1e-3.

Per-core kernel:
  - x is loaded once (4 DMAs, fp16, issued before the weights so the
    transposes are never starved) and PE-transposed into 4 per-octet
    tiles xts[oct][(dc,a)=128, (b,h,w)=1024].
  - Weights are packed ON THE HOST into 144 compact K=32 stationary
    blocks [32, 128]: block (kk,oct,fb) holds capsules
    n=kk*32+oct*8+{2fb,2fb+1}; rows = partitions 32fb..32fb+32 of the
    (dc,a) layout, so the SBUF image is a dense [128, 36*128] tile and
    the matmuls use tile_position=(32*fb, 0) row-strips (the PE contracts
    only K=32 rows; LDWEIGHTS on different row groups overlaps in-flight
    matmuls).
  - Matmul orientation: weights STATIONARY (128-col LDWEIGHTS), moving
    operand streams the im2col window straight out of xts via a strided
    3-free-dim AP [2 batches, 14, 14] = 392 rows -> one PSUM bank.
    No im2col compaction pass at all.
  - fp16 matmul = 1 cyc/row on the PE; f32 accumulate in PSUM.
  - PSUM->SBUF copies (f32 -> fp16 cast) alternate DVE/ACT; one
    contiguous ~800 KB output DMA per (kk,oct), alternating the sync
    HWDGE ring and the gpsimd SWDGE path.
  - Host: upcast fp16 -> f32 + transpose into the reference layout.
"""

import numpy as np

B, H, W, C, A = 4, 16, 16, 32, 16
KS = 3
OH = OW = 14
NCAP = KS * KS * C          # 288 capsules
FTOT = 512                  # filter*atom
NCORES = 8
FPC = FTOT // NCORES        # 64 output features per core
POS = B * OH * OW           # 784 output positions
NPOS = B * H * W            # 1024 input positions
NBLKC = KS * KS * 4         # 36 column-blocks (kk, oct) of 128 cols

_NC_CACHE = {}


def _build_nc():
    import concourse.mybir as mybir
    import concourse.tile as tile
    from concourse import bacc, masks

    f16 = mybir.dt.float16
    f32 = mybir.dt.float32

    nc = bacc.Bacc(None, target_bir_lowering=False)
    x_d = nc.declare_dram_parameter("x", [NPOS, 512], f16, isOutput=False)
    w_d = nc.declare_dram_parameter("wp", [128, NBLKC * 128], f16,
                                    isOutput=False)
    # out[kk, oct, col(=hi*64+f), fb, pos] fp16; host decodes
    # n = kk*32 + oct*8 + 2*fb + hi, feature = core*64 + f.
    o_d = nc.declare_dram_parameter("out", [KS * KS, 4, 128, 4, POS], f16,
                                    isOutput=True)

    with tile.TileContext(nc) as tc:
        with (
            tc.tile_pool(name="const", bufs=1) as constp,
            tc.tile_pool(name="big", bufs=1) as bigp,
            tc.tile_pool(name="stage", bufs=3) as stagep,
            tc.tile_pool(name="psum", bufs=8, space="PSUM") as psump,
        ):
            ident = constp.tile([128, 128], f16, tag="ident")
            masks.make_identity(nc, ident[:])

            # ---- x first: the transposes gate everything downstream
            xsrc = x_d.rearrange("(t s p) c -> t p s c", t=4, p=128)
            x_sbs = [
                bigp.tile([128, 2 * 512], f16, tag=f"x_sb{t}",
                          name=f"x_sb{t}")
                for t in range(4)
            ]
            for t in range(4):
                nc.sync.dma_start(
                    x_sbs[t][:].rearrange("p (s c) -> p s c", s=2), xsrc[t]
                )

            # ---- weights: compact [128, 36*128]; kk=0 chunk first.
            # Block (kk,oct,fb) = wsb[32*fb:32*(fb+1), blk*128:(blk+1)*128],
            # blk = kk*4+oct.
            wsbA = bigp.tile([128, 4 * 128], f16, tag="wsbA")
            wsbB = bigp.tile([128, 32 * 128], f16, tag="wsbB")
            nc.scalar.dma_start(wsbA[:], w_d[:, 0:4 * 128])
            nc.scalar.dma_start(wsbB[:], w_d[:, 4 * 128:NBLKC * 128])

            def wblock(blk, fb):
                if blk < 4:
                    return wsbA[32 * fb:32 * (fb + 1),
                                blk * 128:(blk + 1) * 128]
                return wsbB[32 * fb:32 * (fb + 1),
                            (blk - 4) * 128:(blk - 3) * 128]

            # ---- xT: PE-transpose x into 4 per-octet tiles [(dc,a), pos]
            xts = [
                bigp.tile([128, NPOS], f16, tag=f"xt{o}", name=f"xt{o}")
                for o in range(4)
            ]
            for oct in range(4):
                for s in range(8):
                    tr = psump.tile([128, 128], f16, tag="mm")
                    nc.tensor.transpose(
                        tr[:],
                        x_sbs[s // 2][
                            :, (s % 2) * 512 + oct * 128:
                            (s % 2) * 512 + (oct + 1) * 128
                        ],
                        ident[:],
                    )
                    dst = xts[oct][:, s * 128:(s + 1) * 128]
                    if (s + oct) % 2 == 0:
                        nc.vector.tensor_copy(dst, tr[:])
                    else:
                        nc.scalar.copy(dst, tr[:])

            xtvs = [
                t[:].rearrange("p (b h w) -> p b h w", b=B, h=H) for t in xts
            ]

            # ---- main stream: 36 (kk,oct) groups x 4 fb row-strips x 2
            # moving windows.  fb inner so consecutive LDWEIGHTS target
            # different PE row groups (pull-ahead under in-flight matmuls).
            HB = 2 * OH * OW  # 392 positions per half (2 batches)
            cp_rot = 0
            it = 0
            for kk in range(KS * KS):
                ki, kj = kk // 3, kk % 3
                for oct in range(4):
                    blk = kk * 4 + oct
                    st = stagep.tile([128, 4 * POS], f16, tag="st")
                    for half in range(2):
                        for fb in range(4):
                            ps = psump.tile([128, 512], f32, tag="mm")
                            mv = xtvs[oct][
                                32 * fb:32 * (fb + 1),
                                2 * half:2 * half + 2,
                                ki:ki + OH, kj:kj + OW,
                            ]
                            nc.tensor.matmul(
                                ps[:, 0:HB], wblock(blk, fb), mv,
                                start=True, stop=True,
                                tile_position=(32 * fb, 0),
                            )
                            dst = st[:, fb * POS + half * HB:
                                     fb * POS + (half + 1) * HB]
                            # alternate PSUM->SBUF cast copies DVE/ACT
                            if cp_rot % 2 == 0:
                                nc.vector.tensor_copy(dst, ps[:, 0:HB])
                            else:
                                nc.scalar.copy(dst, ps[:, 0:HB])
                            cp_rot += 1
                    dma_eng = nc.sync if it % 2 == 0 else nc.gpsimd
                    dma_eng.dma_start(
                        o_d[kk, oct],
                        st[:].rearrange("p (fb q) -> p fb q", fb=4),
                    )
                    it += 1

    nc.compile()
    return nc


def _get_nc():
    if "nc" not in _NC_CACHE:
        _NC_CACHE["nc"] = _build_nc()
    return _NC_CACHE["nc"]


def _in_maps(x, matrix):
    """Full f32 inputs -> per-core fp16 input dicts."""
    x16 = np.ascontiguousarray(
        x.reshape(NPOS, 512).astype(np.float16)
    )
    m16 = matrix.astype(np.float16)  # [288, 16, 512]
    maps = []
    for c in range(NCORES):
        msl = m16[:, :, c * FPC:(c + 1) * FPC]      # [288, 16, 64]
        blk = msl.reshape(KS * KS, 4, 8, 16, FPC)   # [kk, oct, dc, a, f]
        wp = np.zeros((128, NBLKC, 2, FPC), np.float16)
        for dc in range(8):
            hi = dc % 2
            # rows (dc,a) = partitions dc*16..dc*16+16 (= 32*fb + 16*hi + a)
            wp[dc * 16:(dc + 1) * 16].reshape(16, KS * KS, 4, 2, FPC)[
                :, :, :, hi, :
            ] = blk[:, :, dc].transpose(2, 0, 1, 3)
        maps.append({
            "x": x16,
            "wp": np.ascontiguousarray(wp.reshape(128, NBLKC * 128)),
        })
    return maps


def kernel(x, matrix):
    from concourse.bass_utils import run_bass_kernel_spmd

    x = np.ascontiguousarray(x, dtype=np.float32)
    matrix = np.ascontiguousarray(matrix, dtype=np.float32)
    nc = _get_nc()
    r = run_bass_kernel_spmd(nc, _in_maps(x, matrix), list(range(NCORES)))
    # parts[c]: [9, 4, 128, 4, 784] fp16
    arr = np.stack([r.results[c]["out"] for c in range(NCORES)])
    arr = arr.reshape(NCORES, KS * KS, 4, 2, FPC, 4, POS)
    # [core, kk, oct, hi, f, fb, pos] -> [pos, kk, oct, fb, hi, core, f]
    arr = arr.transpose(6, 1, 2, 5, 3, 0, 4)
    full = arr.reshape(POS, NCAP, FTOT).astype(np.float32)
    return np.ascontiguousarray(
        full.reshape(B, OH, OW, NCAP, 32, 16)
    )
